# revision 4
# baseline (speedup 1.0000x reference)
"""Trainium2 Bass kernel for nn_BaseAttention (B=2, N=2048, E=2048, H=16, D=128).

Sharding: 8 cores; core c handles batch b=c//4, head-group hg=c%4 (4 heads).

All-fp8 projections (cost model: fp8 DoubleRow = 0.25 cyc/col per
256-contraction vs 1.0 fp16):
- Chunks 1-3: single-pass e4m3 DoubleRow (weights x64 on host, descaled at
  PSUM eviction). Chunk 0 (positions 0-511, feeding the short-softmax rows)
  runs three DR passes (hi*hi + hi*lo + lo*hi) with e4m3 residual tensors,
  giving ~fp16-class accuracy at 0.75x the fp16 PE cost.
- q/k evict fp16 (QK^T runs fp16); v evicts e4m3 with a 0.25 ones-column.
- exp on ScalarE, bias -ln4, e5m2 out for chunks>=1, fp16 for chunk 0.
  Causal masking: per-chunk A buffers, one-time zero prefixes, [128,128]
  triangle masks (on GPSIMD) for the diagonal strips.
- A@V: fp16 chunk 0, e5m2 x e4m3 DoubleRow otherwise; normalization fused
  into PSUM eviction; attnout e4m3 (x4), rows<256 also fp16.
- Out-projection: rows>=256 fp8 DoubleRowSwInterleave on byte-pair DMA
  transposed attnout (rows per 128-block come out reversed; undone on
  host); rows<256 fp16. DR rows carry x256, removed on host; fp16 output
  upcast on the host.

Emission uses a virtual-clock scheduler: the QK/exp stream (ScalarE-heavy)
is interleaved at ~1us quanta with PE-filler (projection chains, oproj
row-blocks) so no engine head-of-line-blocks the TensorE pipeline.
"""

import sys
import time
from collections import deque

sys.path.insert(0, "/opt/trn_rl_repo")

import numpy as np
import ml_dtypes

import concourse.bass as bass
import concourse.mybir as mybir
import concourse.tile as tile
from concourse import bacc
from concourse.bass_utils import run_bass_kernel_spmd

B, N, E, H = 2, 2048, 2048, 16
D = E // H
HPC = 4
DC = HPC * D
NCORES = 8
P = 128
NCH = 4
ET = 16
EP = ET // 2

F32 = mybir.dt.float32
FP16 = mybir.dt.float16
F8E4 = mybir.dt.float8e4
F8E5 = mybir.dt.float8e5
U16 = mybir.dt.uint16
DRM = mybir.MatmulPerfMode.DoubleRow
DRSWI = mybir.MatmulPerfMode.DoubleRowSwInterleave
EXPF = mybir.ActivationFunctionType.Exp
COPYF = mybir.ActivationFunctionType.Copy

SQD = float(D ** -0.5)
WS = 64.0
B0 = float(-np.log(4.0))
E4 = ml_dtypes.float8_e4m3
E5 = ml_dtypes.float8_e5m2


class Sched:
    """Virtual PE/ACT clocks + PE-filler queue for emission balancing."""

    def __init__(self):
        self.pe = 0.0
        self.act = 0.0
        self.in_flush = False
        self.fill = deque()

    def filler(self, pe_cost, fn):
        self.fill.append((pe_cost, fn))

    def balance(self, slack=1500.0):
        while self.fill and self.pe < self.act + slack:
            c, fn = self.fill.popleft()
            fn()
            self.pe += c

    def sync(self):
        self.pe = self.act = max(self.pe, self.act)

    def take(self, n=1):
        for _ in range(min(n, len(self.fill))):
            c, fn = self.fill.popleft()
            fn()
            self.pe += c

    def flush(self):
        self.in_flush = True
        while self.fill:
            c, fn = self.fill.popleft()
            fn()
            self.pe += c
        self.in_flush = False


def build_nc():
    nc = bacc.Bacc("TRN2", target_bir_lowering=False, debug=False,
                   num_devices=NCORES)

    xT8 = nc.dram_tensor("xT8", [E, N], F8E4, kind="ExternalInput")
    xT8l = nc.dram_tensor("xT8l", [E, 512], F8E4, kind="ExternalInput")
    wq8 = nc.dram_tensor("wq8", [E, DC], F8E4, kind="ExternalInput")
    wk8 = nc.dram_tensor("wk8", [E, DC], F8E4, kind="ExternalInput")
    wv8 = nc.dram_tensor("wv8", [E, DC], F8E4, kind="ExternalInput")
    wq8l = nc.dram_tensor("wq8l", [E, DC], F8E4, kind="ExternalInput")
    wk8l = nc.dram_tensor("wk8l", [E, DC], F8E4, kind="ExternalInput")
    wv8l = nc.dram_tensor("wv8l", [E, DC], F8E4, kind="ExternalInput")
    wo16 = nc.dram_tensor("wo16", [DC, E], FP16, kind="ExternalInput")
    wo8 = nc.dram_tensor("wo8", [2, P, 2, E], F8E4, kind="ExternalInput")
    m16in = nc.dram_tensor("m16in", [P, P], FP16, kind="ExternalInput")
    m5in = nc.dram_tensor("m5in", [P, P], F8E5, kind="ExternalInput")
    out = nc.dram_tensor("out", [N, E], FP16, kind="ExternalOutput")

    xT8_r = xT8.ap().rearrange("(eo p) n -> p eo n", p=P)
    xT8l_r = xT8l.ap().rearrange("(eo p) n -> p eo n", p=P)
    w_r = {}
    for name, tsr in (("q", wq8), ("k", wk8), ("v", wv8),
                      ("ql", wq8l), ("kl", wk8l), ("vl", wv8l)):
        w_r[name] = tsr.ap().rearrange("(eo p) d -> p eo d", p=P)
    wo16_r = wo16.ap().rearrange("(t p) e -> p t e", p=P)
    wo8_r = wo8.ap().rearrange("hp p s e -> p hp s e")

    sch = Sched()

    with tile.TileContext(nc) as tc:
        consts = tc.alloc_tile_pool(name="consts", bufs=1)
        _ll = [consts]
        m16 = consts.tile([P, P], FP16)
        m5 = consts.tile([P, P], F8E5)
        bias0 = consts.tile([P, 1], F32)
        nc.vector.memset(bias0, B0)
        dummy = consts.tile([1, 8], F32)
        nc.vector.memset(dummy, 0.0)
        nc.scalar.activation(out=dummy, in_=dummy, func=EXPF)

        dram = tc.alloc_tile_pool(name="dram", bufs=1, space="DRAM")
        _ll.append(dram)
        attd8 = dram.tile([2, N, 256], F8E4)
        attd16 = dram.tile([HPC, 256, D], FP16)

        big = tc.alloc_tile_pool(name="big", bufs=1)
        _ll.append(big)
        q16s = big.tile([P, HPC, N], FP16)
        k16s = big.tile([P, HPC, N], FP16)
        v16 = big.tile([P, NCH, HPC, D + 2], FP16)
        v8s = big.tile([P, 16, HPC, D + 2], F8E4)
        nc.vector.memset(v16[:, :, :, D:D + 1], 0.25)
        nc.vector.memset(v8s[:, :, :, D:D + 1], 0.25)

        # PSUM: 2x2 qk + 4 shared mm/av/oproj = 8 banks
        qk_ps = tc.alloc_tile_pool(name="qk_ps", bufs=2, space="PSUM")
        mm_ps = tc.alloc_tile_pool(name="mm_ps", bufs=4, space="PSUM")
        av_ps = mm_ps
        _ll += [qk_ps, mm_ps]

        w8pool = tc.alloc_tile_pool(name="w8pool", bufs=1)
        _ll.append(w8pool)
        wq8sb = w8pool.tile([P, ET, DC], F8E4)
        wk8sb = w8pool.tile([P, ET, DC], F8E4)
        wv8sb = w8pool.tile([P, ET, DC], F8E4)
        x8sb = w8pool.tile([P, ET, N], F8E4)

        big2a = tc.alloc_tile_pool(name="big2a", bufs=1)
        _ll.append(big2a)
        outT8 = [big2a.tile([P, N, 2], F8E4, name=f"outT8_{hp}")
                 for hp in range(2)]
        outT16 = big2a.tile([P, HPC, 256], FP16)
        att_pool = tc.alloc_tile_pool(name="att_pool", bufs=4)
        rs_pool = tc.alloc_tile_pool(name="rs_pool", bufs=8)
        op_ev = tc.alloc_tile_pool(name="op_ev", bufs=3)
        _ll += [att_pool, rs_pool, op_ev]

        a5 = {}
        wo_sb = {}
        abuf = {}
        lo = {}

        # ---------------- emission units ----------------
        def proj_chain(kind, t_or_nb, nch):
            """One projection chain; chunk 0 runs 3 split-fp8 DR passes."""
            xs = slice(nch * 512, (nch + 1) * 512)
            ps = mm_ps.tile([P, 512], F32, tag="mmps")
            if kind == "v":
                nb = t_or_nb
                xb = slice(nch * 512 + nb * P, nch * 512 + (nb + 1) * P)
                passes = [(x8sb, xb, wv8sb)]
                if nch == 0:
                    passes += [(x8sb, xb, lo["v"]), (lo["x"], xb, wv8sb)]
                nmm = len(passes) * EP
                i = 0
                for xt, xbs, wt in passes:
                    for ep in range(EP):
                        nc.tensor.matmul(
                            ps, lhsT=xt[:, 2 * ep:2 * ep + 2, xbs],
                            rhs=wt[:, 2 * ep:2 * ep + 2, :],
                            start=(i == 0), stop=(i == nmm - 1),
                            perf_mode=DRM)
                        i += 1
                psr = ps.rearrange("p (h d) -> p h d", h=HPC)
                nc.vector.tensor_scalar_mul(
                    out=v8s[:, nch * 4 + nb, :, :D], in0=psr,
                    scalar1=1.0 / WS)
                if nch == 0:
                    nc.scalar.activation(
                        out=v16[:, nb, :, :D], in_=psr, func=COPYF,
                        scale=1.0 / WS)
                    sch.act += 700
            else:
                whi = wq8sb if kind == "q" else wk8sb
                dst, sc = ((q16s, SQD / WS) if kind == "q"
                           else (k16s, 1.0 / WS))
                t = t_or_nb
                tsl = slice(t * P, (t + 1) * P)
                passes = [(whi, x8sb, xs)]
                if nch == 0:
                    passes += [(lo[kind], x8sb, xs), (whi, lo["x"], slice(0, 512))]
                nmm = len(passes) * EP
                i = 0
                for wt, xt, xss in passes:
                    for ep in range(EP):
                        nc.tensor.matmul(
                            ps, lhsT=wt[:, 2 * ep:2 * ep + 2, tsl],
                            rhs=xt[:, 2 * ep:2 * ep + 2, xss],
                            start=(i == 0), stop=(i == nmm - 1),
                            perf_mode=DRM)
                        i += 1
                nc.vector.tensor_scalar_mul(
                    out=dst[:, t, xs], in0=ps, scalar1=sc)

        def queue_proj_chunk(nch):
            for kind in ("q", "k"):
                for t in range(HPC):
                    sch.filler(860, lambda k=kind, t=t: proj_chain(k, t, nch))
            for nb in range(4):
                sch.filler(860, lambda nb=nb: proj_chain("v", nb, nch))

        def oproj_nb(nb, tail=False):
            ostage = op_ev.tile([P, NCH, 512], FP16, tag="opev")
            for ec in range(NCH):
                ps = mm_ps.tile([P, 512], F32, tag="mmps")
                esl = slice(ec * 512, (ec + 1) * 512)
                if nb < 2:
                    for t in range(HPC):
                        nc.tensor.matmul(
                            ps, lhsT=outT16[:, t, nb * P:(nb + 1) * P],
                            rhs=wo_sb[16][:, t, esl],
                            start=(t == 0), stop=(t == HPC - 1))
                else:
                    for hp in range(2):
                        nc.tensor.matmul(
                            ps, lhsT=outT8[hp][:, nb * P:(nb + 1) * P, :],
                            rhs=wo_sb[8][:, hp, :, esl],
                            start=(hp == 0), stop=(hp == 1), perf_mode=DRSWI)
                if tail and ec % 2 == 1:
                    nc.scalar.activation(out=ostage[:, ec, :], in_=ps,
                                         func=COPYF)
                    sch.act += 570
                else:
                    nc.vector.tensor_copy(out=ostage[:, ec, :], in_=ps)
            nc.sync.dma_start(out=out.ap()[nb * P:(nb + 1) * P, :], in_=ostage)

        def queue_oproj_group(g, half=None):
            nbs = range(4 * g, 4 * g + 4)
            if half is not None:
                nbs = nbs[:2] if half == 0 else nbs[2:]
            for nb in nbs:
                cost = 3500 if nb < 2 else 1750
                sch.filler(cost, lambda nb=nb: oproj_nb(nb))

        def slot(h, ci):
            # chunks 0 and 3 run all four heads' scores before the first
            # A@V, so they need per-head A slots
            return h if ci in (0, 3) else h % 2

        def emit_qk_exp(h, ci):
            BJ = 4 * (ci + 1)
            npairs = BJ // 2
            hs = slot(h, ci)
            ab = abuf[0] if ci == 0 else a5[ci]
            isl_all = slice(ci * 512, (ci + 1) * 512)
            for bjp in range(npairs):
                ps = qk_ps.tile([P, 2, 512], F32, tag="qkps")
                for u in range(2):
                    bj = 2 * bjp + u
                    # diagonal blocks: only query cols >= rr*128 are ever
                    # read by exp -- trim the moving free dim
                    if ci == 0:
                        rr = bj
                    elif bjp >= npairs - 2:
                        rr = (bjp - (npairs - 2)) * 2 + u
                    else:
                        rr = 0
                    nc.tensor.matmul(
                        ps[:, u, rr * P:],
                        lhsT=k16s[:, h, bj * P:(bj + 1) * P],
                        rhs=q16s[:, h, ci * 512 + rr * P:(ci + 1) * 512],
                        start=True, stop=True)
                sch.pe += 430
                if ci == 0:
                    for u in range(2):
                        bj = 2 * bjp + u
                        nc.scalar.activation(
                            out=ab[:, hs, bj, bj * P:], in_=ps[:, u, bj * P:],
                            func=EXPF)
                        sch.act += (512 - bj * P) * 0.833 + 250
                elif bjp < npairs - 2:
                    nc.scalar.activation(
                        out=ab[:, hs, 2 * bjp:2 * bjp + 2, :], in_=ps,
                        func=EXPF, bias=bias0[:, 0:1])
                    sch.act += 1024 * 0.833 + 250
                else:
                    rrb = (bjp - (npairs - 2)) * 2
                    for u in range(2):
                        rr = rrb + u
                        nc.scalar.activation(
                            out=ab[:, hs, 2 * bjp + u, rr * P:],
                            in_=ps[:, u, rr * P:],
                            func=EXPF, bias=bias0[:, 0:1])
                        sch.act += (512 - rr * P) * 0.833 + 250
                sch.balance()
            msk = m16 if ci == 0 else m5
            for rr in range(4):
                bj = BJ - 4 + rr
                ssl = slice(rr * P, (rr + 1) * P)
                nc.gpsimd.tensor_mul(out=ab[:, hs, bj, ssl],
                                     in0=ab[:, hs, bj, ssl], in1=msk)

        def emit_av(h, ci):
            BJ = 4 * (ci + 1)
            npairs = BJ // 2
            hs = slot(h, ci)
            hp, hc = divmod(h, 2)
            ab = abuf[0] if ci == 0 else a5[ci]
            isl_all = slice(ci * 512, (ci + 1) * 512)
            att8 = att_pool.tile([P, 4, D], F8E4, tag="att8")
            for ibp in range(2):
                avp = av_ps.tile([P, 2, 256], F32, tag="mmps")
                for u in range(2):
                    ib = 2 * ibp + u
                    isl = slice(ib * P, (ib + 1) * P)
                    if ci == 0:
                        for bj in range(ib + 1):
                            nc.tensor.matmul(
                                avp[:, u, :D + 1],
                                lhsT=ab[:, hs, bj, isl],
                                rhs=v16[:, bj, h, :D + 1],
                                start=(bj == 0), stop=(bj == ib))
                    else:
                        pairs = npairs - (1 if ib < 2 else 0)
                        for bjp in range(pairs):
                            nc.tensor.matmul(
                                avp[:, u, :D + 1],
                                lhsT=ab[:, hs, 2 * bjp:2 * bjp + 2, isl],
                                rhs=v8s[:, 2 * bjp:2 * bjp + 2, h, :D + 1],
                                start=(bjp == 0), stop=(bjp == pairs - 1),
                                perf_mode=DRM)
                sch.pe += 500
                sch.take(2)
                rs = rs_pool.tile([P, 2, 1], F32, tag="rs")
                nc.vector.reciprocal_approx_fast(
                    out=rs, in_=avp[:, :, D:D + 1])
                nc.vector.tensor_mul(
                    out=att8[:, 2 * ibp:2 * ibp + 2, :],
                    in0=avp[:, :, :D], in1=rs.broadcast_to([P, 2, D]))
                if ci == 0 and ibp == 0:
                    rsq = rs_pool.tile([P, 2, 1], F32, tag="rsq")
                    nc.vector.tensor_scalar_mul(out=rsq, in0=rs, scalar1=0.25)
                    a16st = rs_pool.tile([P, 2, D], FP16, tag="a16st")
                    nc.vector.tensor_mul(
                        out=a16st, in0=avp[:, :, :D],
                        in1=rsq.broadcast_to([P, 2, D]))
                    nc.sync.dma_start(
                        out=attd16[h].rearrange("(io p) d -> p io d", p=P),
                        in_=a16st)
                    nc.sync.dma_start_transpose(
                        out=outT16[:, h, :], in_=attd16[h])
            nc.sync.dma_start(
                out=attd8[hp, isl_all, hc * P:(hc + 1) * P].rearrange(
                    "(io p) d -> p io d", p=P),
                in_=att8)

        def emit_transpose(hp, ci):
            isl_all = slice(ci * 512, (ci + 1) * 512)
            nc.sync.dma_start_transpose(
                out=outT8[hp][:, isl_all, :].bitcast(U16),
                in_=attd8[hp, isl_all, :].bitcast(U16))
            sch.take(2)

        # -------- chunk 0 (split-fp8) interleaved with wave 0 --------
        with tc.tile_pool(name="c0pool", bufs=1) as c0pool:
            lo["q"] = c0pool.tile([P, ET, DC], F8E4, name="wq8lsb")
            lo["k"] = c0pool.tile([P, ET, DC], F8E4, name="wk8lsb")
            lo["v"] = c0pool.tile([P, ET, DC], F8E4, name="wv8lsb")
            lo["x"] = c0pool.tile([P, ET, 512], F8E4, name="x8lsb")
            abuf[0] = c0pool.tile([P, 4, 4, 512], FP16, name="a16")

            tsl = slice(0, P)
            for gs in (slice(0, 8), slice(8, 16)):
                nc.sync.dma_start(out=x8sb[:, gs, 0:512],
                                  in_=xT8_r[:, gs, 0:512])
                nc.sync.dma_start(out=wq8sb[:, gs, tsl],
                                  in_=w_r["q"][:, gs, tsl])
                nc.sync.dma_start(out=lo["q"][:, gs, tsl],
                                  in_=w_r["ql"][:, gs, tsl])
            nc.sync.dma_start(out=lo["x"], in_=xT8l_r)
            nc.sync.dma_start(out=wk8sb[:, :, 0:P], in_=w_r["k"][:, :, 0:P])
            nc.sync.dma_start(out=lo["k"][:, :, 0:P], in_=w_r["kl"][:, :, 0:P])
            nc.sync.dma_start(out=m16, in_=m16in.ap())
            nc.sync.dma_start(out=m5, in_=m5in.ap())
            for t in (1, 2, 3):
                tsl = slice(t * P, (t + 1) * P)
                for hi, lo_ in (("q", "ql"), ("k", "kl")):
                    dsth = wq8sb if hi == "q" else wk8sb
                    dstl = lo["q"] if hi == "q" else lo["k"]
                    nc.sync.dma_start(out=dsth[:, :, tsl],
                                      in_=w_r[hi][:, :, tsl])
                    nc.sync.dma_start(out=dstl[:, :, tsl],
                                      in_=w_r[lo_][:, :, tsl])
            nc.sync.dma_start(out=wv8sb, in_=w_r["v"])
            nc.sync.dma_start(out=lo["v"], in_=w_r["vl"])
            nc.sync.dma_start(out=x8sb[:, :, 512:1024],
                              in_=xT8_r[:, :, 512:1024])
            for g in (2, 3):
                nc.sync.dma_start(out=x8sb[:, :, g * 512:(g + 1) * 512],
                                  in_=xT8_r[:, :, g * 512:(g + 1) * 512])

            for h in (0, 1, 2, 3):
                proj_chain("q", h, 0)
                proj_chain("k", h, 0)
                sch.pe += 5160
                emit_qk_exp(h, 0)
            for nb in range(4):
                proj_chain("v", nb, 0)
                sch.pe += 2580
            queue_proj_chunk(1)
            for h in (0, 1):
                emit_av(h, 0)
            emit_transpose(0, 0)
            for h in (2, 3):
                emit_av(h, 0)
            emit_transpose(1, 0)
            sch.flush()

        big2b = tc.alloc_tile_pool(name="big2b", bufs=1)
        _ll.append(big2b)
        for ci in (1, 2, 3):
            nslot = 4 if ci == 3 else 2
            a5[ci] = big2b.tile([P, nslot, 4 * (ci + 1), 512], F8E5,
                                name=f"a5_{ci}")
            bjn = 4 * (ci + 1)
            for hs in range(nslot):
                nc.vector.memset(a5[ci][:, hs, bjn - 3, 0:P], 0.0)
                nc.vector.memset(a5[ci][:, hs, bjn - 1, 2 * P:3 * P], 0.0)
        wo_sb[16] = big2b.tile([P, HPC, E], FP16, name="wo16sb")
        wo_sb[8] = big2b.tile([P, 2, 2, E], F8E4, name="wo8sb")
        nc.sync.dma_start(out=wo_sb[8], in_=wo8_r)
        for t in range(HPC):
            nc.sync.dma_start(out=wo_sb[16][:, t, :], in_=wo16_r[:, t, :])

        # ---------------- waves 1-3 ----------------
        for ci in (1, 2):
            emit_qk_exp(0, ci)
            queue_proj_chunk(ci + 1)
            emit_qk_exp(1, ci)
            emit_av(0, ci)
            emit_qk_exp(2, ci)
            queue_oproj_group(ci - 1)
            emit_av(1, ci)
            emit_transpose(0, ci)
            emit_qk_exp(3, ci)
            emit_av(2, ci)
            emit_av(3, ci)
            emit_transpose(1, ci)
            if ci == 2:
                sch.flush()     # chunk-3 chains must land before its scores
                emit_qk_exp(0, 3)
                emit_qk_exp(1, 3)
            sch.flush()
        queue_oproj_group(2, half=0)
        emit_av(0, 3)
        emit_av(1, 3)
        emit_transpose(0, 3)
        emit_qk_exp(2, 3)
        emit_av(2, 3)
        queue_oproj_group(2, half=1)
        emit_qk_exp(3, 3)
        emit_av(3, 3)
        emit_transpose(1, 3)
        sch.flush()
        for nb in range(12, 16):
            oproj_nb(nb, tail=True)

        for _pl in reversed(_ll):
            _pl.release()

    nc.compile()
    return nc


def make_in_maps(x, Wq, Wkv, Wout):
    x = np.asarray(x, dtype=np.float32)
    Wq = np.asarray(Wq, dtype=np.float32)
    Wkv = np.asarray(Wkv, dtype=np.float32)
    Wout = np.asarray(Wout, dtype=np.float32)

    jj = np.arange(P)[:, None]
    ii = np.arange(P)[None, :]
    tri = (ii >= jj)
    m16 = tri.astype(np.float16)
    m5 = tri.astype(E5)

    in_maps = []
    for c in range(NCORES):
        b, hg = divmod(c, 4)
        sl = slice(hg * DC, (hg + 1) * DC)
        xT = np.ascontiguousarray(x[b].T)
        x8 = xT.astype(E4)
        x8l = (xT[:, :512] - x8[:, :512].astype(np.float32)).astype(E4)
        wq = np.ascontiguousarray(Wq[sl, :].T) * WS
        wk = np.ascontiguousarray(Wkv[sl, :].T) * WS
        wv = np.ascontiguousarray(Wkv[E + sl.start:E + sl.stop, :].T) * WS
        woT = np.ascontiguousarray(Wout[:, sl].T)
        wq8 = wq.astype(E4)
        wk8 = wk.astype(E4)
        wv8 = wv.astype(E4)
        wo8 = np.empty((2, P, 2, E), dtype=E4)
        for hp in range(2):
            blk = woT[hp * 256:(hp + 1) * 256] * WS
            wo8[hp, :, 0, :] = blk[0::2].astype(E4)
            wo8[hp, :, 1, :] = blk[1::2].astype(E4)
        in_maps.append({
            "xT8": x8,
            "xT8l": x8l,
            "wq8": wq8,
            "wk8": wk8,
            "wv8": wv8,
            "wq8l": (wq - wq8.astype(np.float32)).astype(E4),
            "wk8l": (wk - wk8.astype(np.float32)).astype(E4),
            "wv8l": (wv - wv8.astype(np.float32)).astype(E4),
            "wo16": woT.astype(np.float16),
            "wo8": wo8,
            "m16in": m16,
            "m5in": m5,
        })
    return in_maps


_NC_CACHE = []


def _get_nc():
    if not _NC_CACHE:
        _NC_CACHE.append(build_nc())
    return _NC_CACHE[0]


def _run(in_maps):
    nc = _get_nc()
    return run_bass_kernel_spmd(nc, in_maps, core_ids=list(range(NCORES)))


def kernel(x, Wq, Wkv, Wout):
    in_maps = make_in_maps(x, Wq, Wkv, Wout)
    res = _run(in_maps)
    out = np.zeros((B, N, E), dtype=np.float32)
    for c in range(NCORES):
        out[c // 4] += res.results[c]["out"].astype(np.float32)
    # SwInterleave oproj emits rows reversed within each 128-block (nb >= 2)
    out[:, 256:, :] = out[:, 256:, :].reshape(B, 14, P, E)[:, :, ::-1, :].reshape(
        B, N - 256, E)
    out[:, 256:, :] *= 1.0 / 256.0
    return out


if __name__ == "__main__":
    t0 = time.time()
    _get_nc()
    print(f"build+compile: {time.time() - t0:.1f}s")
